# revision 33
# baseline (speedup 1.0000x reference)
"""RGCN basis-decomposed message passing on 8 TRN2 NeuronCores (v2).

Math: reference computes, per relation r:
    support_r = A @ x[:, :, r]      (A = sparse adjacency from edges, same for all r)
    out = concat_r(support_r) @ W   (W[r] = sum_b w_rel[r,b] * w_bases[b])
By linearity:  out = A @ (sum_r x[:,:,r] @ W_r) = A @ y,   y = x_flat @ w_perm.

Distribution (8 cores):
  - nodes sharded contiguously: core c owns nodes [c*NS, (c+1)*NS)
  - phase A (per core): y_shard = x_shard @ w_perm  (bf16 matmuls on TensorE)
  - AllGather y (bf16) -> every core holds the full y table in DRAM
  - phase C (per core): edges with dst in the shard, grouped into 128-node dst
    groups. Per group: two dma_gather calls (lo/hi int16 windows) fetch the
    y[src] rows for all the group's edge slots; one batched DVE is_equal
    builds every tile's one-hot in transposed layout [P, 128 dst, T tiles];
    one batched DVE multiply computes msg = gbuf * w_bcast; per-tile
    matmuls lhsT=onehot[:, :, t], rhs=msg[:, t, :] accumulate into the
    group's PSUM bank.

The Q7 gather descriptor path (~3ns per 256B descriptor with all 4 SWDGE
queues busy, ~327us for ~103k descriptors/core) is the wall; the batched
DVE ops (~185us) and matmuls (~95us) hide under it. v2 spent 300us of
Scalar on per-tile scaled copies and 200us of DVE on per-tile one-hots;
batching them per group removed both from the critical path.
"""

import math
import sys
from contextlib import ExitStack

for _p in ("/opt/trn_rl_repo",):
    if _p not in sys.path:
        sys.path.insert(0, _p)

import ml_dtypes
import numpy as np

import concourse.bacc as bacc
import concourse.bass as bass
import concourse.mybir as mybir
from concourse import library_config
from concourse.bass_utils import run_bass_kernel_spmd

F32 = mybir.dt.float32
BF16 = mybir.dt.bfloat16
I16 = mybir.dt.int16
NPBF16 = ml_dtypes.bfloat16
P = 128
COPY = mybir.ActivationFunctionType.Copy


class Cfg:
    def __init__(self, N, E, D=64, R=8, C=8):
        self.N, self.E, self.D, self.R, self.C = N, E, D, R, C
        assert N % C == 0
        self.NS = N // C                     # nodes per core
        self.G = math.ceil(self.NS / P)      # 128-node dst groups per core
        self.NS_PAD = self.G * P
        self.NTAB = C * self.NS_PAD          # gather-table rows (all-gathered y)
        # y is all-gathered in two group-chunks (chunk-major layout) = the two
        # int16 gather windows. The lo chunk is as small as the 32768-row hi
        # window limit allows, so its AllGather finishes early and the big hi
        # AllGather hides under the lo-window gathers.
        gmax_hi = (32768 // (C * P))
        self.glo = max(1, self.G - gmax_hi) if self.G > 1 else 1
        if self.G > self.glo:
            self.ag_chunks = [(0, self.glo), (self.glo, self.G)]
            self.half = 1
        else:
            self.ag_chunks = [(0, self.G)]
            self.half = 1
        self.NAG = len(self.ag_chunks)
        self.WLO = C * P * self.glo                   # rows in window lo
        assert self.WLO <= 32768
        assert self.NTAB - self.WLO <= 32768
        self.K = D * R // P                  # 128-row contraction chunks
        assert D * R % P == 0


def plan_and_pack(cfg, x, edge_src, edge_dst, edge_weight, w_bases, w_rel):
    """Host preprocessing (the sharding step). Returns (plan, in_maps)."""
    C, NS, G, D = cfg.C, cfg.NS, cfg.G, cfg.D
    NS_PAD = cfg.NS_PAD

    es = edge_src.astype(np.int64)
    ed = edge_dst.astype(np.int64)
    ew = edge_weight.astype(np.float32)

    # dense weights: w[r] = sum_b w_rel[r,b] w_bases[b];  w_perm[(i,r), o]
    w = np.einsum("rb,bio->rio", w_rel.astype(np.float64),
                  w_bases.astype(np.float64)).astype(np.float32)
    w_perm = np.ascontiguousarray(w.transpose(1, 0, 2).reshape(D * cfg.R, D))
    wp = np.ascontiguousarray(
        w_perm.reshape(cfg.K, P, D).transpose(1, 0, 2).reshape(P, cfg.K * D)
    ).astype(NPBF16)

    # chunk-major gather-table row for global node v = (c, l):
    # chunk i holds groups [a_i, b_i); within chunk i the row is
    # base_i + (c*P + l%P) * w_i + (l//P - a_i),  w_i = b_i - a_i
    src_c, src_l = es // NS, es % NS
    g_src, p_src = src_l // P, src_l % P
    chunk_of = np.zeros(G, np.int64)
    base = np.zeros(len(cfg.ag_chunks) + 1, np.int64)
    for i, (a, b) in enumerate(cfg.ag_chunks):
        chunk_of[a:b] = i
        base[i + 1] = base[i] + C * P * (b - a)
    ci = chunk_of[g_src]
    a_i = np.array([cfg.ag_chunks[i][0] for i in range(cfg.NAG)])[ci]
    w_i = np.array([b - a for (a, b) in cfg.ag_chunks])[ci]
    trow = base[ci] + (src_c * P + p_src) * w_i + (g_src - a_i)
    dst_c, dst_l = ed // NS, ed % NS
    g_of, n_loc = dst_l // P, dst_l % P
    WLO = cfg.WLO
    is_hi = (trow >= WLO).astype(np.int64)

    # bucket edges by (core, group, window); each window's run is padded to
    # whole 128-slot tiles so every gather call has all-valid indices
    key = (dst_c * G + g_of) * 2 + is_hi
    order = np.argsort(key, kind="stable")
    bounds = np.searchsorted(key[order], np.arange(C * G * 2 + 1))
    cnt = (bounds[1:] - bounds[:-1]).reshape(C, G, 2)

    Tj = np.maximum(1, np.ceil(cnt.max(axis=0) / P).astype(int))  # [G, 2]
    # descriptors per (g, window): only up to the max real count across
    # cores, rounded to the 16-slot idx-wrap granularity (the remaining
    # tail slots of the last tile are never gathered; their weights are 0
    # and gbuf is memset once so they contribute exact zeros)
    NIDX = np.maximum(16, (np.ceil(cnt.max(axis=0) / 16) * 16).astype(int))
    NIDX = np.minimum(NIDX, Tj * P)
    T = Tj.sum(axis=1)
    cum = np.zeros(G + 1, np.int64)
    cum[1:] = np.cumsum(T)
    TT = int(cum[-1])
    T_lo = Tj[:, 0]
    T_hi = Tj[:, 1]

    t_s, n_s, w_s = trow[order], n_loc[order], ew[order]

    Tmax = int(T.max())
    iota_rep = np.ascontiguousarray(np.broadcast_to(
        np.arange(P, dtype=np.float32)[None, :, None], (P, P, Tmax)
    )).astype(NPBF16).reshape(P, P * Tmax)

    in_maps = []
    for c in range(C):
        dstloc = np.full((P, TT), 999.0, np.float32)
        wgt = np.zeros((P, TT), np.float32)
        idxw = np.zeros((P, 8 * TT), np.int16)
        for g in range(G):
            for j in range(2):
                b = (c * G + g) * 2 + j
                lo_, hi_ = bounds[b], bounds[b + 1]
                n = hi_ - lo_
                t0 = cum[g] + (T_lo[g] if j else 0)
                L = Tj[g, j] * P
                vals = np.zeros(L, np.int64)
                vals[:n] = t_s[lo_:hi_] - (WLO if j else 0)
                # wrapped int16 layout: position i -> [i%16, 8*t0 + i//16],
                # replicated to every 16-partition group (q7 tx/rx pairs
                # read their own group, selected by queue_num)
                idxw[:, 8 * t0: 8 * (t0 + Tj[g, j])] = np.tile(
                    vals.astype(np.int16).reshape(L // 16, 16).T, (8, 1))
                s = np.arange(n)
                pp, tt = s % P, t0 + s // P
                dstloc[pp, tt] = n_s[lo_:hi_]
                wgt[pp, tt] = w_s[lo_:hi_]

        # x^T layout [D*R, NS_PAD] zero-padded, bf16
        xs = x[c * NS:(c + 1) * NS].reshape(NS, D * cfg.R)
        xT = np.zeros((D * cfg.R, NS_PAD), NPBF16)
        xT[:, :NS] = xs.T.astype(NPBF16)
        in_maps.append({
            "xT": xT, "wp": wp.copy(), "gidx": idxw,
            "dstloc": dstloc.astype(NPBF16),
            "wgt": wgt, "iota": iota_rep.copy(),
        })

    plan = {"T": T.tolist(), "cum": cum.tolist(), "TT": TT,
            "T_lo": T_lo.tolist(), "T_hi": T_hi.tolist(),
            "NIDX": NIDX.tolist(), "Tmax": Tmax}
    return plan, in_maps


def build_nc(cfg, plan, nps=6, nslot=6):
    C, G, D, K = cfg.C, cfg.G, cfg.D, cfg.K
    NS_PAD, TT = cfg.NS_PAD, plan["TT"]
    T, cum = plan["T"], plan["cum"]
    T_lo, T_hi = plan["T_lo"], plan["T_hi"]
    NIDX = plan["NIDX"]
    Tmax = max(T)

    nc = bacc.Bacc("TRN2", num_swdge_queues=4)

    xT_d = nc.declare_dram_parameter("xT", [K * P, NS_PAD], BF16, isOutput=False)
    wp_d = nc.declare_dram_parameter("wp", [P, K * D], BF16, isOutput=False)
    gidx_d = nc.declare_dram_parameter("gidx", [P, 8 * TT], I16, isOutput=False)
    dstloc_d = nc.declare_dram_parameter("dstloc", [P, TT], BF16, isOutput=False)
    wgt_d = nc.declare_dram_parameter("wgt", [P, TT], F32, isOutput=False)
    iota_d = nc.declare_dram_parameter("iota", [P, P * Tmax], BF16, isOutput=False)
    out_d = nc.declare_dram_parameter("out", [P, G * D], F32, isOutput=True)

    # chunk-major: y_own = concat_i [P, w_i*D]; y_all = concat_i [C*P, w_i*D]
    y_own = nc.dram_tensor("y_own", [P * G * D], F32)
    y_all = nc.dram_tensor("y_all", [C * P * G * D], F32, addr_space="Shared")

    NCH = min(8, G)  # xT node-range chunks
    NAG = cfg.NAG
    ag_chunks = cfg.ag_chunks
    obase = np.zeros(NAG + 1, np.int64)   # element offsets into y_own
    abase = np.zeros(NAG + 1, np.int64)   # element offsets into y_all
    for i, (a, b) in enumerate(ag_chunks):
        obase[i + 1] = obase[i] + P * (b - a) * D
        abase[i + 1] = abase[i] + C * P * (b - a) * D

    with ExitStack() as top:
        sem = top.enter_context
        s_wp = sem(nc.semaphore("s_wp"))
        s_xt = [sem(nc.semaphore(f"s_xt{i}")) for i in range(NCH)]
        s_meta = sem(nc.semaphore("s_meta"))
        s_mmA = sem(nc.semaphore("s_mmA"))
        s_yA = sem(nc.semaphore("s_yA"))
        s_ydma_c = [sem(nc.semaphore(f"s_ydma{i}")) for i in range(NAG)]
        s_cc = sem(nc.semaphore("s_cc"))
        s_g = [[sem(nc.semaphore(f"s_g{i}_{d}")) for d in range(2)]
               for i in range(4)]
        s_act = sem(nc.semaphore("s_act"))
        s_gz = sem(nc.semaphore("s_gz"))
        s_m = sem(nc.semaphore("s_m"))
        s_mm = sem(nc.semaphore("s_mm"))
        s_po = sem(nc.semaphore("s_po"))
        s_od = sem(nc.semaphore("s_od"))
        # -------- single block: phase A feeds phase C (no block barrier) -----
        if True:
            pa = top
            gidx_sb = pa.enter_context(nc.sbuf_tensor("gidx_sb", [P, 8 * TT], I16))
            dstloc_sb = pa.enter_context(nc.sbuf_tensor("dstloc_sb", [P, TT], BF16))
            wgt_sb = pa.enter_context(nc.sbuf_tensor("wgt_sb", [P, TT], F32))
            iota_sb = pa.enter_context(nc.sbuf_tensor("iota_sb", [P, P, Tmax], BF16))
            out_sb = pa.enter_context(nc.sbuf_tensor("out_sb", [P, G, D], F32))
            xT_sb = pa.enter_context(nc.sbuf_tensor("xT_sb", [P, K, NS_PAD], BF16))
            wp_sb = pa.enter_context(nc.sbuf_tensor("wp_sb", [P, K * D], BF16))
            y_sb = pa.enter_context(nc.sbuf_tensor("y_sb", [P, G, D], F32))
            psA = [pa.enter_context(nc.psum_tensor(f"psA{i}", [P, D], F32))
                   for i in range(2)]
            step = (G + NCH - 1) // NCH
            nt_chunks = [(i * step, min(G, (i + 1) * step)) for i in range(NCH)]
            nt_chunks = [(a, b) for (a, b) in nt_chunks if b > a]
            NCH = len(nt_chunks)


            def phaseA_sync(sync):
                sync.dma_start(out=wp_sb[:], in_=wp_d[:]).then_inc(s_wp, 16)
                for ci, (a, b) in enumerate(nt_chunks):
                    sync.dma_start(
                        out=xT_sb[:, :, a * P:b * P],
                        in_=xT_d.rearrange("(k p) n -> p k n", p=P)[:, :, a * P:b * P],
                    ).then_inc(s_xt[ci], 16)
                for i, (a, b) in enumerate(ag_chunks):
                    sync.wait_ge(s_yA, b)
                    sync.dma_start(
                        out=y_own[int(obase[i]):int(obase[i + 1])].rearrange(
                            "(p w) -> p w", p=P),
                        in_=y_sb[:, a:b, :],
                    ).then_inc(s_ydma_c[i], 16)
                    if i == 0:
                        # metadata loads deferred past the xT/y-own critical
                        # path: not needed until the first phase-C scales
                        sync.dma_start(out=gidx_sb[:],
                                       in_=gidx_d[:]).then_inc(s_meta, 16)
                        sync.dma_start(out=dstloc_sb[:],
                                       in_=dstloc_d[:]).then_inc(s_meta, 16)
                        sync.dma_start(out=wgt_sb[:],
                                       in_=wgt_d[:]).then_inc(s_meta, 16)
                        sync.dma_start(
                            out=iota_sb[:],
                            in_=iota_d.rearrange("p (j t) -> p j t", j=P),
                        ).then_inc(s_meta, 16)

            def phaseA_tensor(tensor):
                tensor.wait_ge(s_wp, 16)
                for ci, (a, b) in enumerate(nt_chunks):
                    tensor.wait_ge(s_xt[ci], 16)
                    for nt in range(a, b):
                        if nt >= 2:
                            tensor.wait_ge(s_yA, nt - 1)
                        for k in range(K):
                            mm = tensor.matmul(
                                psA[nt % 2][:],
                                xT_sb[:, k, nt * P:(nt + 1) * P],
                                wp_sb[:, k * D:(k + 1) * D],
                                start=(k == 0), stop=(k == K - 1),
                            )
                        mm.then_inc(s_mmA, 1)

            def phaseA_vector(vector):
                for nt in range(G):
                    vector.wait_ge(s_mmA, nt + 1)
                    vector.tensor_copy(
                        out=y_sb[:, nt, :], in_=psA[nt % 2][:]
                    ).then_inc(s_yA, 1)

        # ---------------- phase C: out = A @ y ----------------
        y_rows = y_all.rearrange("(q d) -> q d", d=D)
        HALF = cfg.half
        if True:
            pc = pa
            sb = pc.enter_context
            gbuf = [sb(nc.sbuf_tensor(f"gbuf{i}", [P, Tmax, D], F32))
                    for i in range(nslot)]
            mbuf = [sb(nc.sbuf_tensor(f"mbuf{i}", [P, P, Tmax], BF16))
                    for i in range(nslot)]
            msg = [sb(nc.sbuf_tensor(f"msg{i}", [P, Tmax, D], BF16))
                   for i in range(nslot)]
            ps = [sb(nc.psum_tensor(f"psC{i}", [P, D], F32)) for i in range(nps)]
            blockC = pc.enter_context(nc.Block())


            @blockC.gpsimd
            def _(gpsimd):
                gpsimd.load_library(library_config.mlp)
                gpsimd.wait_ge(s_gz, nslot)
                for i in range(NAG):
                    gpsimd.wait_ge(s_ydma_c[i], 16)
                    gpsimd.collective_compute(
                        "AllGather",
                        mybir.AluOpType.bypass,
                        replica_groups=[list(range(C))],
                        ins=[y_own[int(obase[i]):int(obase[i + 1])].opt()],
                        outs=[y_all[int(abase[i]):int(abase[i + 1])].opt()],
                    ).then_inc(s_cc)
                hi_window = (y_rows[cfg.WLO:cfg.NTAB, :]
                             if cfg.NTAB > cfg.WLO else y_rows[0:cfg.NTAB, :])

                def gather_lo(g):
                    n = NIDX[g][0]
                    gpsimd.dma_gather(
                        gbuf[g % nslot][:, 0:T_lo[g], :],
                        y_rows[0:cfg.WLO, :],
                        gidx_sb[:, 8 * cum[g]: 8 * cum[g] + n // 16],
                        n, n, D,
                        single_packet=False, queue_num=(2 * g) % 4,
                    ).then_inc(s_g[(2 * g) % 4][(g // 2) % 2], 16)

                def gather_hi(g):
                    n = NIDX[g][1]
                    t0 = cum[g] + T_lo[g]
                    gpsimd.dma_gather(
                        gbuf[g % nslot][:, T_lo[g]:T[g], :],
                        hi_window,
                        gidx_sb[:, 8 * t0: 8 * t0 + n // 16],
                        n, n, D,
                        single_packet=False, queue_num=(2 * g + 1) % 4,
                    ).then_inc(s_g[(2 * g + 1) % 4][(g // 2) % 2], 16)

                # prefix: first nslot lo-gathers run before any hi gather so
                # the hi-window AllGather hides under them
                npre = min(nslot, G)
                gpsimd.wait_ge(s_cc, HALF)
                for g in range(npre):
                    gather_lo(g)
                gpsimd.wait_ge(s_cc, NAG)
                for g in range(npre):
                    gather_hi(g)
                for g in range(npre, G):
                    if g >= nslot:
                        gpsimd.wait_ge(s_act, cum[g - nslot + 1])
                    gather_lo(g)
                    gather_hi(g)

            @blockC.scalar
            def _(scalar):
                for g in range(G):
                    scalar.wait_ge(s_mm, g + 1)
                    scalar.copy(
                        out_sb[:, g, :], ps[g % nps][:]
                    ).then_inc(s_po, 1)

            @blockC.vector
            def _(vector):
                # memsets first: s_gz gates the gpsimd block (collectives +
                # gathers); running them before the phase-A psum copies lets
                # the AllGather rendezvous start ~40us earlier
                for i in range(nslot):
                    vector.memset(gbuf[i][:], 0.0).then_inc(s_gz, 1)
                phaseA_vector(vector)
                vector.wait_ge(s_meta, 64)

                def onehot(g):
                    # batched transposed one-hot [P, 128 dst, T[g] tiles]
                    if g >= nslot:
                        vector.wait_ge(s_mm, g - nslot + 1)
                    vector.tensor_tensor(
                        out=mbuf[g % nslot][:, :, 0:T[g]],
                        in0=dstloc_sb[:, cum[g]:cum[g + 1]].rearrange(
                            "p (x t) -> p x t", x=1).to_broadcast([P, P, T[g]]),
                        in1=iota_sb[:, :, 0:T[g]],
                        op=mybir.AluOpType.is_equal,
                    ).then_inc(s_m, T[g])

                def msgpass(g):
                    # batched weight multiply msg = gbuf * w, split per
                    # gather window (v2's wait discipline): the lo half runs
                    # as soon as the lo gather lands, advancing s_act so the
                    # Pool pacing gate unblocks earlier
                    tl = int(T_lo[g])
                    vector.wait_ge(s_g[(2 * g) % 4][(g // 2) % 2],
                                   16 * (g // 4 + 1))
                    vector.tensor_tensor(
                        out=msg[g % nslot][:, 0:tl, :],
                        in0=gbuf[g % nslot][:, 0:tl, :],
                        in1=wgt_sb[:, cum[g]:cum[g] + tl].to_broadcast(
                            [P, tl, D]),
                        op=mybir.AluOpType.mult,
                    ).then_inc(s_act, tl)
                    th = int(T[g]) - tl
                    vector.wait_ge(s_g[(2 * g + 1) % 4][(g // 2) % 2],
                                   16 * (g // 4 + 1))
                    vector.tensor_tensor(
                        out=msg[g % nslot][:, tl:T[g], :],
                        in0=gbuf[g % nslot][:, tl:T[g], :],
                        in1=wgt_sb[:, cum[g] + tl:cum[g + 1]].to_broadcast(
                            [P, th, D]),
                        op=mybir.AluOpType.mult,
                    ).then_inc(s_act, th)

                onehot(0)
                for g in range(1, G):
                    onehot(g)
                    msgpass(g - 1)
                msgpass(G - 1)

            @blockC.tensor
            def _(tensor):
                phaseA_tensor(tensor)
                for g in range(G):
                    if g >= nps:
                        tensor.wait_ge(s_po, g - nps + 1)
                    tensor.wait_ge(s_m, cum[g + 1])
                    tensor.wait_ge(s_act, cum[g + 1])
                    for t in range(T[g]):
                        mm = tensor.matmul(
                            ps[g % nps][:],
                            mbuf[g % nslot][:, :, t],
                            msg[g % nslot][:, t, :],
                            start=(t == 0), stop=(t == T[g] - 1),
                        )
                    mm.then_inc(s_mm, 1)

            @blockC.sync
            def _(sync):
                phaseA_sync(sync)
                ostep = (G + 7) // 8
                nod = 0
                for a in range(0, G, ostep):
                    b = min(G, a + ostep)
                    sync.wait_ge(s_po, b)
                    sync.dma_start(
                        out=out_d[:, a * D:b * D], in_=out_sb[:, a:b, :]
                    ).then_inc(s_od, 16)
                    nod += 16
                sync.wait_ge(s_od, nod)

    nc.compile()
    return nc


def gnn_kernel(x, edge_src, edge_dst, edge_weight, w_bases, w_rel,
               cfg=None, trace=False):
    if cfg is None:
        cfg = Cfg(N=50000, E=800000)
    plan, in_maps = plan_and_pack(cfg, np.asarray(x), np.asarray(edge_src),
                                  np.asarray(edge_dst), np.asarray(edge_weight),
                                  np.asarray(w_bases), np.asarray(w_rel))
    nc = build_nc(cfg, plan)
    res = run_bass_kernel_spmd(nc, in_maps, list(range(cfg.C)), trace=trace)
    outs = res.results
    D, G, NS = cfg.D, cfg.G, cfg.NS
    full = np.empty((cfg.N, D), np.float32)
    for c in range(cfg.C):
        o = outs[c]["out"].reshape(P, G, D).transpose(1, 0, 2).reshape(cfg.NS_PAD, D)
        full[c * NS:(c + 1) * NS] = o[:NS]
    return full, res


def kernel(x, edge_src, edge_dst, edge_weight, w_bases, w_rel):
    """Full inputs in, full output out. Shards across 8 NeuronCores inside."""
    cfg = Cfg(N=50000, E=800000)
    plan, in_maps = plan_and_pack(cfg, np.asarray(x), np.asarray(edge_src),
                                  np.asarray(edge_dst), np.asarray(edge_weight),
                                  np.asarray(w_bases), np.asarray(w_rel))
    nc = build_nc(cfg, plan)
    res = run_bass_kernel_spmd(nc, in_maps, list(range(cfg.C)))
    outs = res.results
    D, G, NS = cfg.D, cfg.G, cfg.NS
    full = np.empty((cfg.N, D), np.float32)
    for c in range(cfg.C):
        o = outs[c]["out"].reshape(P, G, D).transpose(1, 0, 2).reshape(cfg.NS_PAD, D)
        full[c * NS:(c + 1) * NS] = o[:NS]
    return full


